# revision 3
# baseline (speedup 1.0000x reference)
"""Contrastive-head loss kernel for Trainium2 (8 NeuronCores, data parallel).

Math (per row i of similarity [B, N], select [B, N] in {0,1}, T = 0.1):
    pos    = sum(sim * [sel==1]) / max(count(sel==1), 1)
    pl     = pos / T
    lse    = log(exp(pl) + sum_{sel==0} exp(sim / T))
    loss_i = lse - pl
    out    = mean_i loss_i

Device strategy: shard rows across 8 cores (512 rows/core = 4 tiles of 128
partitions). Each core streams column-chunks of sim/sel and computes three
per-row partial reductions per chunk:
    cnt = sum(sel)                  ACT Copy(int32->f32) accum_out
    E   = sum(exp(10*sim)[sel==0])  ACT Exp, then DVE stt (is_equal 0)*e
    sK  = sum(sim * sel)            DVE stt (kf*1)*sim
Chunk partials land in stats [128, 3*nsub]; the host finishes the per-row
log/exp and the global mean in float64 (4096 rows - negligible).

exp inputs are 10*sim in [-88, 88] (no -inf masking needed) and every masked
sum is computed directly, so there is no large-term cancellation anywhere.

Race-free DMA synchronization (validated against CoreSim's race detector):
DMA-completion semaphore increments from concurrently in-flight DMAs are NOT
ordered (16 SDMA engines each +1 as their slice lands; engine skew can mix
increments across DMAs), so waiting on a partial prefix count is a race. The
schedule therefore gives every SBUF slot its own pair of semaphores (ksem_b,
ssem_b) with at most ONE outstanding DMA per semaphore: slot reuse is gated
on DVE consumption of the previous occupant (SP wait_ge on dve_sem), which
guarantees the consumer-side waits (ksem_b >= 16*(n+1)) are on that lane's
cumulative count. kf/e sub-buffers rotate mod 4, ordered through the
act_sem -> dve_sem chain. The DVE elementwise outputs (values dead, only
accum_out matters) alias their own in-place slots to avoid scratch WAW.

The chunk schedule is tapered at both ends (512-col first/last chunks) so
the pipeline fills fast and drains fast; body chunks are 2048 cols (1 MiB
DMAs, 8 KB descriptors) which sustain ~404 GB/s/core on HW (93% of the
435 GB/s SBUF-fabric ceiling).

Raw Bass (no TileContext): this walrus build rejects instructions with >2
sync waits; every dependency is a standalone wait_ge or a single-sem DMA inc.
"""

import sys
from contextlib import ExitStack

for _p in ("/opt/trn_rl_repo",):
    if _p not in sys.path:
        sys.path.insert(0, _p)

import numpy as np

import concourse.bass as bass
import concourse.mybir as mybir
from concourse.bass_utils import run_bass_kernel_spmd

B, N = 4096, 8192
NCORES = 8
RB = B // NCORES  # rows per core
P = 128  # SBUF partitions
NT = RB // P  # row tiles per core
INV_T = 10.0

SUBW = 2048  # compute sub-chunk width; also the SBUF slot width
NSLOT = 8

# Per-tile chunk widths (each <= SUBW, sums to N). Tapered at the start of
# tile 0 (fast pipeline fill) and the end of tile 3 (fast drain).
TILE_SPLITS = {
    0: [512, 512, 1024, 2048, 2048, 2048],
    1: [2048] * 4,
    2: [2048] * 4,
    3: [2048, 2048, 2048, 1024, 512, 512],
}


def _schedule(reps):
    """[(tile, col_off, width), ...] per chunk; one chunk = one DMA pair."""
    chunks = []
    for _ in range(reps):
        for t in range(NT):
            off = 0
            for w in TILE_SPLITS[t]:
                chunks.append((t, off, w))
                off += w
    return chunks


NSUB = len(_schedule(1))  # chunks per rep = stat triples per rep


def _build_nc(reps=1):
    """reps > 1 repeats the full streaming pipeline on-device (same input
    reread from HBM each rep); used only for differential HW timing."""
    chunks = _schedule(reps)
    ntot = len(chunks)
    nc = bass.Bass(trn_type="TRN2")
    sim = nc.dram_tensor("similarity", [RB, N], mybir.dt.float32, kind="ExternalInput")
    sel = nc.dram_tensor("select", [RB, N], mybir.dt.int32, kind="ExternalInput")
    stats = nc.dram_tensor("stats", [P, 3 * NSUB], mybir.dt.float32, kind="ExternalOutput")

    with ExitStack() as ctx:
        s_sl = ctx.enter_context(
            nc.sbuf_tensor("s_sl", [P, NSLOT * SUBW], mybir.dt.float32)
        )
        k_sl = ctx.enter_context(
            nc.sbuf_tensor("k_sl", [P, NSLOT * SUBW], mybir.dt.int32)
        )
        kf_sl = ctx.enter_context(nc.sbuf_tensor("kf_sl", [P, 4 * SUBW], mybir.dt.float32))
        e_sl = ctx.enter_context(nc.sbuf_tensor("e_sl", [P, 4 * SUBW], mybir.dt.float32))
        stats_t = ctx.enter_context(
            nc.sbuf_tensor("stats_t", [P, 3 * NSUB], mybir.dt.float32)
        )
        ksem = [ctx.enter_context(nc.semaphore(f"ksem{b}")) for b in range(NSLOT)]
        ssem = [ctx.enter_context(nc.semaphore(f"ssem{b}")) for b in range(NSLOT)]
        stats_sem = ctx.enter_context(nc.semaphore("stats_sem"))
        act_sem = ctx.enter_context(nc.semaphore("act_sem"))
        dve_sem = ctx.enter_context(nc.semaphore("dve_sem"))
        block = ctx.enter_context(nc.Block())

        @block.sync
        def _(sync):
            for j, (t, off, w) in enumerate(chunks):
                b = j % NSLOT
                if j >= NSLOT:
                    # slot b free once chunk j-NSLOT fully consumed (DVE op2)
                    sync.wait_ge(dve_sem, 2 * (j - NSLOT) + 2)
                rows = slice(t * P, (t + 1) * P)
                cols = slice(off, off + w)
                sb = b * SUBW
                sync.dma_start(out=k_sl[:, sb : sb + w], in_=sel[rows, cols]).then_inc(
                    ksem[b], 16
                )
                sync.dma_start(out=s_sl[:, sb : sb + w], in_=sim[rows, cols]).then_inc(
                    ssem[b], 16
                )
            # split stats store: the first NSUB-4 chunks' triples are final
            # once chunk ntot-5 is consumed, so that store overlaps the tail
            # chunks' compute; only the last 4 triples go in the final store.
            # Both stores inc one cumulative stats_sem (wait >= 32 is the
            # lane's total - race-free).
            cut = 3 * (NSUB - 4)
            sync.wait_ge(dve_sem, 2 * (ntot - 4))
            sync.dma_start(out=stats[:, :cut], in_=stats_t[:, :cut]).then_inc(
                stats_sem, 16
            )
            sync.wait_ge(dve_sem, 2 * ntot)
            sync.dma_start(out=stats[:, cut:], in_=stats_t[:, cut:]).then_inc(
                stats_sem, 16
            )
            sync.wait_ge(stats_sem, 32)
            for b in range(NSLOT):
                uses = len(range(b, ntot, NSLOT))
                sync.wait_ge(ksem[b], 16 * uses)
                sync.wait_ge(ssem[b], 16 * uses)

        @block.scalar
        def _(scalar):
            for j, (t, off, w) in enumerate(chunks):
                b = j % NSLOT
                jc = j % NSUB
                sb = b * SUBW
                fs = (j % 4) * SUBW
                if j >= 4:
                    # kf/e sub-slot free once chunk j-4's DVE ops retired
                    scalar.wait_ge(dve_sem, 2 * (j - 4) + 2)
                scalar.wait_ge(ksem[b], 16 * (j // NSLOT + 1))
                scalar.activation(
                    kf_sl[:, fs : fs + w],
                    k_sl[:, sb : sb + w],
                    mybir.ActivationFunctionType.Copy,
                    accum_out=stats_t[:, 3 * jc : 3 * jc + 1],
                ).then_inc(act_sem, 1)
                scalar.wait_ge(ssem[b], 16 * (j // NSLOT + 1))
                scalar.activation(
                    e_sl[:, fs : fs + w],
                    s_sl[:, sb : sb + w],
                    mybir.ActivationFunctionType.Exp,
                    scale=INV_T,
                ).then_inc(act_sem, 1)

        @block.vector
        def _(vector):
            for j, (t, off, w) in enumerate(chunks):
                b = j % NSLOT
                jc = j % NSUB
                sb = b * SUBW
                fs = (j % 4) * SUBW
                # act_sem >= 2j+2: copy_j and exp_j done (implies DMAs landed)
                vector.wait_ge(act_sem, 2 * j + 2)
                vector.scalar_tensor_tensor(
                    out=e_sl[:, fs : fs + w],
                    in0=kf_sl[:, fs : fs + w],
                    scalar=0.0,
                    in1=e_sl[:, fs : fs + w],
                    op0=mybir.AluOpType.is_equal,
                    op1=mybir.AluOpType.mult,
                    accum_out=stats_t[:, 3 * jc + 1 : 3 * jc + 2],
                ).then_inc(dve_sem, 1)
                vector.scalar_tensor_tensor(
                    out=kf_sl[:, fs : fs + w],
                    in0=kf_sl[:, fs : fs + w],
                    scalar=1.0,
                    in1=s_sl[:, sb : sb + w],
                    op0=mybir.AluOpType.mult,
                    op1=mybir.AluOpType.mult,
                    accum_out=stats_t[:, 3 * jc + 2 : 3 * jc + 3],
                ).then_inc(dve_sem, 1)

    return nc


_TILE_OF_SUB = [t for (t, _o, _w) in _schedule(1)]


def _finish_rows(stats_core):
    """stats_core [P, 3*NSUB] f32 -> per-row losses [RB] (f64)."""
    st = np.asarray(stats_core, dtype=np.float64).reshape(P, NSUB, 3)
    cnt = np.zeros((P, NT))
    E = np.zeros((P, NT))
    sK = np.zeros((P, NT))
    for j, t in enumerate(_TILE_OF_SUB):
        cnt[:, t] += st[:, j, 0]
        E[:, t] += st[:, j, 1]
        sK[:, t] += st[:, j, 2]
    pos = sK / np.maximum(np.round(cnt), 1.0)
    pl = INV_T * pos
    loss = np.log(E + np.exp(pl)) - pl  # [P, NT]
    return loss.T.reshape(RB)  # global row within core = t * P + p


def kernel(similarity, select, _run_kwargs=None):
    similarity = np.ascontiguousarray(similarity, dtype=np.float32)
    select = np.ascontiguousarray(select, dtype=np.int32)
    assert similarity.shape == (B, N) and select.shape == (B, N)

    nc = _build_nc()
    in_maps = [
        {
            "similarity": similarity[i * RB : (i + 1) * RB],
            "select": select[i * RB : (i + 1) * RB],
        }
        for i in range(NCORES)
    ]
    res = run_bass_kernel_spmd(nc, in_maps, list(range(NCORES)), **(_run_kwargs or {}))

    losses = np.empty((B,), dtype=np.float64)
    for i in range(NCORES):
        losses[i * RB : (i + 1) * RB] = _finish_rows(res.results[i]["stats"])
    out = np.asarray(losses.mean(), dtype=np.float32)
    if _run_kwargs is not None:
        return out, res
    return out
